# revision 20
# baseline (speedup 1.0000x reference)
"""Trainium2 Bass kernel for nn_CodecDecoder — 8-core SPMD, sequence-sharded.

Strategy: 8 cores = 2 batches x 4 sequence-quarters. Sliding-window attention
(ws<=16) and small causal convs mean each core only needs a small left halo of
the sequence; every core computes its slice of the final output independently
(zero collectives). Activations are feature-major (features on SBUF
partitions); matmul inputs are bf16, residual stream is f32.
"""

import math
import numpy as np
import ml_dtypes

import concourse.bass as bass
import concourse.tile as tile
from concourse import mybir
from concourse.bass_utils import run_bass_kernel_spmd
from concourse.masks import make_identity

F32 = mybir.dt.float32
BF16 = mybir.dt.bfloat16
BF16_NP = ml_dtypes.bfloat16
AF = mybir.ActivationFunctionType
ALU = mybir.AluOpType

DIM, H, HD, HID = 1024, 8, 128, 4096
EPS = 0.01
SEM_N, SEM_D, NAC, FSQ, SPECIAL, PATCH = 8192, 256, 36, 21, 2, 240
WINDOWS = [2, 4, 8, 16]
B, T0 = 2, 256
NCORE = 8
PADL = 8                      # left zero-pad columns in every stage buffer
LS = [83, 156, 292, 550]      # per-core local token counts per stage
WS = [PADL + l for l in LS]   # stage buffer widths: 91, 164, 300, 558
DUP = [5, 10, 17]             # upsample input col offset per stage transition
OUTC0 = PADL + 38             # first real out-conv column (46)
KIN_T = 3                     # in-conv K tiles (292 -> 384)
IW = 144                      # scores i-window width per j-chunk (128 + 16)


def _g(stage, c):
    """Global token index of local column PADL for seq-quarter c (may be <0)."""
    return [64 * c - 19, 128 * c - 28, 256 * c - 36, 512 * c - 38][stage]


def _chunks(W, step=512):
    return [(n, min(n + step, W)) for n in range(0, W, step)]


# ---------------------------------------------------------------------------
# host-side weight prep
# ---------------------------------------------------------------------------

def _wn(v, g):
    v = np.asarray(v, np.float32)
    g = np.asarray(g, np.float32)
    n = np.sqrt((v * v).sum(axis=(1, 2), keepdims=True))
    return (g * v / np.maximum(n, 1e-8)).astype(np.float32)


def _lhsT(W):
    """(M, K) weight -> lhsT blocked (Mt, 128, Kt, 128) bf16.
    A[mt, p, kt, m] = W[mt*128+m, kt*128+p]; SBUF tile [:, kt, :] is the
    (K=128, M=128) stationary operand for out = W @ act."""
    W = np.asarray(W, np.float32)
    M, K = W.shape
    Mt, Kt = -(-M // 128), -(-K // 128)
    Wp = np.zeros((Mt * 128, Kt * 128), np.float32)
    Wp[:M, :K] = W
    A = Wp.reshape(Mt, 128, Kt, 128).transpose(0, 3, 2, 1)
    return np.ascontiguousarray(A).astype(BF16_NP)


def _prep_shared(params):
    ins = {}
    st = params["stages"]
    for l in range(8):
        s, li = divmod(l, 2)
        p = {k: np.asarray(v, np.float32) for k, v in st[s][li].items()}
        an, fn = p["an"], p["fn"]
        ins[f"wq{l}"] = _lhsT(p["wq"] * an[None, :])
        ins[f"wk{l}"] = _lhsT(p["wk"] * an[None, :])
        ins[f"wo{l}"] = _lhsT(p["ascale"][:, None] * p["wo"])
        wv = (p["wv"] * an[None, :]).astype(np.float32)
        ins[f"wv{l}"] = np.ascontiguousarray(
            wv.T.reshape(8, 128, DIM)).astype(BF16_NP)
        ins[f"w1{l}"] = _lhsT(p["w1"] * fn[None, :])
        ins[f"w3{l}"] = _lhsT(p["w3"] * fn[None, :])
        ins[f"w2{l}"] = _lhsT(p["fscale"][:, None] * p["w2"])
    # jax.lax.conv is correlation (no flip): out[t] = sum_r w[r] x[t-(k-1)+r]
    w_in = _wn(params["in_v"], params["in_g"])          # (1024, 292, 3)
    ins["win"] = np.stack([_lhsT(w_in[:, :, r]) for r in range(3)])
    for s in range(3):
        wu = _wn(params["ups"][s]["v"], params["ups"][s]["g"])  # (in, out, 4)
        ins[f"wup{s}"] = np.stack([_lhsT(wu[:, :, r].T) for r in range(4)])
    w_out = _wn(params["out_v"], params["out_g"])       # (240, 1024, 7)
    ins["wout"] = np.stack([_lhsT(w_out[:, :, r]) for r in range(7)])
    return ins


def _prep_eb(c):
    """Multiplicative exp-bias masks. eb1: generic j-chunk; eb0: chunk 0 with
    global j>=0 masking (per seq-quarter c). [s, jj, h, ii'] layout."""
    jj = np.arange(128, dtype=np.float64)[:, None]
    ii = np.arange(IW, dtype=np.float64)[None, :]
    eb0 = np.zeros((4, 128, H, IW), np.float32)
    eb1 = np.zeros((4, 128, H, IW), np.float32)
    d = jj - ii  # j - i
    for s, ws in enumerate(WINDOWS):
        for h in range(H):
            slope = 2.0 ** (-8.0 / H * (h + 1))
            ok = (d <= 0) & (d >= -ws)
            e = np.where(ok, np.exp(slope * d), 0.0)
            eb1[s, :, h, :] = e
            ok0 = ok & (jj >= PADL - _g(s, c))
            eb0[s, :, h, :] = np.where(ok0, np.exp(slope * d), 0.0)
    return eb0.astype(BF16_NP), eb1.astype(BF16_NP)


def _prep_percore(codes, params):
    codes = np.asarray(codes)
    tab = np.asarray(params["sem_table"], np.float32) / np.maximum(
        np.asarray(params["sem_usage"], np.float32)[:, None], 1e-8)
    sem = np.clip(codes[:, :, 0] - SPECIAL, 0, SEM_N - 1).astype(np.int64)
    se = tab[sem]                                            # (B, T0, 256)
    ac = (codes[:, :, 1:] - SPECIAL).astype(np.float32) * (2.0 / (FSQ - 1)) - 1.0
    emb = np.concatenate([se, ac], -1).transpose(0, 2, 1)    # (B, 292, T0)
    per_core = []
    ebs = [_prep_eb(c) for c in range(4)]
    for core in range(NCORE):
        b, c = divmod(core, 4)
        g0 = _g(0, c)
        xe = np.zeros((KIN_T * 128, WS[0]), np.float32)
        lo_p = PADL - 2
        for p in range(lo_p, WS[0]):
            t = g0 + p - PADL
            if 0 <= t < T0:
                xe[:292, p] = emb[b, :, t]
        per_core.append({
            "xe": xe.reshape(KIN_T, 128, WS[0]).astype(BF16_NP),
            "eb0": ebs[c][0],
            "eb1": ebs[c][1],
        })
    return per_core


# ---------------------------------------------------------------------------
# IR post-pass: this walrus build encodes at most ONE sync-wait per
# instruction; hoist extra waits onto same-engine NoOps inserted before.
# ---------------------------------------------------------------------------

def _split_multiwait(nc):
    n = 0
    for f in nc.m.functions:
        for bb in f.blocks:
            insts = list(bb.instructions)
            out = []
            changed = False
            for inst in insts:
                si = inst.sync_info
                if si is not None and si.on_wait and len(si.on_wait) > 1:
                    waits = list(si.on_wait)
                    for k, w in enumerate(waits[:-1]):
                        nop = mybir.InstNoOp(name=f"{inst.name}-ws{k}",
                                             ins=[], outs=[])
                        nop.engine = inst.engine
                        nop.sync_info = mybir.SyncInfo(on_wait=[w], on_update=[])
                        out.append(nop)
                    si.on_wait = [waits[-1]]
                    n += 1
                    changed = True
                out.append(inst)
            if changed:
                bb.instructions[:] = out
    return n


# ---------------------------------------------------------------------------
# device program
# ---------------------------------------------------------------------------

def _build(n_layers=8, dbg_stage=None):
    """Build the SPMD program. n_layers<8 truncates after that many
    transformer layers and dumps the residual x instead of the final output
    (debug). dbg_stage pins which stage's x is dumped (defaults to the stage
    of the last layer built)."""
    nc = bass.Bass("TRN2", target_bir_lowering=False, debug=False,
                   num_devices=NCORE)
    D = {}

    def din(name, shape):
        D[name] = nc.dram_tensor(name, shape, BF16, kind="ExternalInput")
        return D[name]

    din("xe", (KIN_T, 128, WS[0]))
    din("eb0", (4, 128, H, IW))
    din("eb1", (4, 128, H, IW))
    for l in range(8):
        din(f"wq{l}", (8, 128, 8, 128))
        din(f"wk{l}", (8, 128, 8, 128))
        din(f"wo{l}", (8, 128, 8, 128))
        din(f"wv{l}", (8, 128, DIM))
        din(f"w1{l}", (32, 128, 8, 128))
        din(f"w3{l}", (32, 128, 8, 128))
        din(f"w2{l}", (8, 128, 32, 128))
    din("win", (3, 8, 128, KIN_T, 128))
    for s in range(3):
        din(f"wup{s}", (4, 8, 128, 8, 128))
    din("wout", (7, 2, 128, 8, 128))

    full = n_layers >= 8
    if full:
        out_d = nc.dram_tensor("out", (2, 128, 512), F32, kind="ExternalOutput")
    else:
        if dbg_stage is None:
            # after n layers: odd n -> mid-stage; even n -> post-upsample
            dbg_stage = min(3, n_layers // 2)
        out_d = nc.dram_tensor("out", (8, 128, WS[dbg_stage]), F32,
                               kind="ExternalOutput")

    with tile.TileContext(nc) as tc:
        from contextlib import ExitStack
        with ExitStack() as ctx:
            sb = ctx.enter_context(tc.tile_pool(name="sb", bufs=1))
            wp = ctx.enter_context(tc.tile_pool(name="wp", bufs=1))
            cst = ctx.enter_context(tc.tile_pool(name="cst", bufs=1))

            ident = cst.tile([128, 128], BF16, name="ident")
            make_identity(nc, ident)
            ones = cst.tile([128, 128], BF16, name="ones")
            nc.vector.memset(ones, 1.0)
            epsb = cst.tile([128, 1], F32, name="epsb")
            nc.vector.memset(epsb, float(EPS))
            epsbq = cst.tile([128, 1], F32, name="epsbq")
            nc.vector.memset(epsbq, float(EPS * HD))

            def xtiles(s):
                W = WS[s]
                tag = f"xres{s % 2}"
                xs = [sb.tile([128, W], F32, name=f"x{s}_{k}", tag=tag, bufs=8)
                      for k in range(8)]
                for t in xs:
                    nc.vector.memset(t[:, 0:PADL], 0.0)
                return xs

            # ---------------- embed + in conv ----------------
            xe = [sb.tile([128, WS[0]], BF16, name=f"xe{k}", tag="xe",
                          bufs=KIN_T) for k in range(KIN_T)]
            for k in range(KIN_T):
                nc.sync.dma_start(xe[k], D["xe"][k])

            x = xtiles(0)
            with tc.tile_pool(name="pconv", bufs=1, space="PSUM") as pp:
                for mt in range(8):
                    ps = pp.tile([128, LS[0]], F32, name="ps_in", tag="pin",
                                 bufs=2)
                    first = True
                    for r in range(3):
                        wt = wp.tile([128, KIN_T, 128], BF16, name="winw",
                                     tag="winw", bufs=4)
                        nc.sync.dma_start(wt, D["win"][r, mt])
                        for kt in range(KIN_T):
                            nc.tensor.matmul(
                                ps, wt[:, kt, :],
                                xe[kt][:, PADL - 2 + r: PADL - 2 + r + LS[0]],
                                start=first, stop=(r == 2 and kt == KIN_T - 1))
                            first = False
                    nc.vector.tensor_copy(out=x[mt][:, PADL:], in_=ps)

            # ---------------- helpers ----------------
            def norm_r(xt, W, pool, extra=1.0, ptag="pss", pbufs=2):
                """r = 1/sqrt(extra*(mean_d(x^2)+EPS)), broadcast (128, W) f32."""
                xsq = [sb.tile([128, W], BF16, name="xsq", tag="xsq", bufs=8)
                       for _ in range(8)]
                for k in range(8):
                    nc.scalar.activation(out=xsq[k], in_=xt[k],
                                         func=AF.Square)
                sq = sb.tile([128, W], F32, name="sq", tag="rt", bufs=3)
                for n0, n1 in _chunks(W):
                    ps = pool.tile([128, 512], F32, name="pss", tag=ptag,
                                   bufs=pbufs)
                    for k in range(8):
                        nc.tensor.matmul(ps[:, :n1 - n0], ones,
                                         xsq[k][:, n0:n1],
                                         start=(k == 0), stop=(k == 7))
                    nc.scalar.activation(out=sq[:, n0:n1], in_=ps[:, :n1 - n0],
                                         func=AF.Sqrt, scale=extra / DIM,
                                         bias=epsbq if extra != 1.0 else epsb)
                r = sb.tile([128, W], F32, name="r", tag="rt", bufs=3)
                nc.vector.reciprocal(out=r, in_=sq)
                return r

            def proj(pool, wname, src, W, tag):
                """out[f] = W' @ src, feature-major, returns 8 bf16 tiles."""
                dst = [sb.tile([128, W], BF16, name=f"{tag}{mt}", tag=tag,
                               bufs=8) for mt in range(8)]
                for mt in range(8):
                    wt = wp.tile([128, 8, 128], BF16, name=f"w_{tag}",
                                 tag=f"w_{tag}", bufs=3)
                    nc.sync.dma_start(wt, D[wname][mt])
                    for n0, n1 in _chunks(W):
                        ps = pool.tile([128, 512], F32, name="pp", tag="pp",
                                       bufs=3)
                        for kt in range(8):
                            nc.tensor.matmul(ps[:, :n1 - n0], wt[:, kt, :],
                                             src[kt][:, n0:n1],
                                             start=(kt == 0), stop=(kt == 7))
                        nc.vector.tensor_copy(out=dst[mt][:, n0:n1],
                                              in_=ps[:, :n1 - n0])
                return dst

            def attn(s, l, x, eb0t, eb1t):
                W = WS[s]
                NCJ = -(-W // 128)
                with tc.tile_pool(name="pN", bufs=1, space="PSUM") as pN:
                    wv = wp.tile([128, 8, DIM], BF16, name="wv", tag="wv",
                                 bufs=1)
                    nc.sync.dma_start(wv, D[f"wv{l}"][:])
                    r = norm_r(x, W, pN)
                    xn = [sb.tile([128, W], BF16, name="xn", tag="xn", bufs=8)
                          for _ in range(8)]
                    for k in range(8):
                        nc.vector.tensor_tensor(out=xn[k], in0=x[k], in1=r,
                                                op=ALU.mult)
                    q = proj(pN, f"wq{l}", xn, W, "q")
                    kk = proj(pN, f"wk{l}", xn, W, "k")
                    # V token-major with appended ones column — issued before
                    # the q/k norms so PE has dense work while DVE norms run
                    vaug = sb.tile([128, NCJ, 8, 129], BF16, name="vaug",
                                   tag="vaug", bufs=1,
                                   padded_shape=[128, 5, 8, 129])
                    nc.vector.memset(vaug[:, :, :, 128:129], 1.0)
                    for cj in range(NCJ):
                        j0, j1 = 128 * cj, min(128 * cj + 128, W)
                        jw = j1 - j0
                        pv = pN.tile([128, DIM], F32, name="pv", tag="pv",
                                     bufs=1)
                        for n0, n1 in ((0, 512), (512, 1024)):
                            for kt in range(8):
                                nc.tensor.matmul(pv[:jw, n0:n1],
                                                 xn[kt][:, j0:j1],
                                                 wv[:, kt, n0:n1],
                                                 start=(kt == 0),
                                                 stop=(kt == 7))
                        nc.vector.tensor_copy(
                            out=vaug[0:jw, cj, :, 0:128],
                            in_=pv[:jw].rearrange("p (h d) -> p h d", h=8))
                    rq = norm_r(q, W, pN, extra=float(HD))
                    rk = norm_r(kk, W, pN)
                    for t in range(8):
                        nc.vector.tensor_tensor(out=q[t], in0=q[t], in1=rq,
                                                op=ALU.mult)
                        nc.vector.tensor_tensor(out=kk[t], in0=kk[t], in1=rk,
                                                op=ALU.mult)

                # scores -> exp -> masked
                expT = sb.tile([128, NCJ, 8, IW], BF16, name="expT",
                               tag="expT", bufs=1,
                               padded_shape=[128, 5, 8, IW])
                with tc.tile_pool(name="pS", bufs=1, space="PSUM") as pS:
                    for cj in range(NCJ):
                        j0, j1 = 128 * cj, min(128 * cj + 128, W)
                        jw = j1 - j0
                        i0, i1 = 128 * cj, min(128 * cj + IW, W)
                        iw = i1 - i0
                        ebt = eb0t if cj == 0 else eb1t
                        for h in range(8):
                            ps = pS.tile([128, IW], F32, name="ps", tag="ps",
                                         bufs=6)
                            nc.tensor.matmul(ps[:jw, :iw], kk[h][:, j0:j1],
                                             q[h][:, i0:i1],
                                             start=True, stop=True)
                            se = sb.tile([128, IW], BF16, name="se", tag="se",
                                         bufs=4)
                            nc.scalar.activation(out=se[:jw, :iw],
                                                 in_=ps[:jw, :iw], func=AF.Exp)
                            nc.vector.tensor_tensor(
                                out=expT[0:jw, cj, h, 0:iw],
                                in0=se[:jw, :iw], in1=ebt[0:jw, h, 0:iw],
                                op=ALU.mult)

                # AV (+denominator via ones column) -> divide -> transpose
                afeat = [sb.tile([128, W], BF16, name="af", tag="afeat",
                                 bufs=8) for _ in range(8)]
                with tc.tile_pool(name="pV", bufs=1, space="PSUM") as pV:
                    for ci in range(NCJ):
                        i0 = 128 * ci
                        iw = min(128, W - i0)
                        atok = sb.tile([128, 8, HD], BF16, name="atok",
                                       tag="atok", bufs=2)
                        for h in range(8):
                            po = pV.tile([128, 129], F32, name="po", tag="po",
                                         bufs=4)
                            jw = min(128, W - i0)
                            if ci > 0:
                                jpw = min(128, W - 128 * (ci - 1))
                                tw = min(16, W - i0)
                                nc.tensor.matmul(
                                    po[0:tw, :],
                                    expT[0:jpw, ci - 1, h, 128:128 + tw],
                                    vaug[0:jpw, ci - 1, h, :],
                                    start=True, stop=False)
                                nc.tensor.matmul(po[0:iw, :],
                                                 expT[0:jw, ci, h, 0:iw],
                                                 vaug[0:jw, ci, h, :],
                                                 start=False, stop=True)
                            else:
                                nc.tensor.matmul(po[0:iw, :],
                                                 expT[0:jw, ci, h, 0:iw],
                                                 vaug[0:jw, ci, h, :],
                                                 start=True, stop=True)
                            den = sb.tile([128, 2], F32, name="den", tag="den",
                                          bufs=4)
                            nc.vector.tensor_scalar_max(out=den[:, 0:1],
                                                        in0=po[:, 128:129],
                                                        scalar1=1e-30)
                            nc.vector.reciprocal(out=den[:, 1:2],
                                                 in_=den[:, 0:1])
                            nc.vector.tensor_scalar_mul(out=atok[:, h, :],
                                                        in0=po[:, 0:128],
                                                        scalar1=den[:, 1:2])
                            pt = pV.tile([128, 128], BF16, name="pt", tag="pt",
                                         bufs=4)
                            nc.tensor.transpose(pt, atok[:, h, :], ident)
                            nc.scalar.copy(out=afeat[h][:, i0:i0 + iw],
                                           in_=pt[:, 0:iw])

                # wo + residual
                with tc.tile_pool(name="pW", bufs=1, space="PSUM") as pW:
                    for mt in range(8):
                        wt = wp.tile([128, 8, 128], BF16, name="w_o",
                                     tag="w_q", bufs=3)
                        nc.sync.dma_start(wt, D[f"wo{l}"][mt])
                        for n0, n1 in _chunks(W):
                            ps = pW.tile([128, 512], F32, name="pw", tag="pw",
                                         bufs=3)
                            for kt in range(8):
                                nc.tensor.matmul(ps[:, :n1 - n0], wt[:, kt, :],
                                                 afeat[kt][:, n0:n1],
                                                 start=(kt == 0),
                                                 stop=(kt == 7))
                            nc.vector.tensor_tensor(out=x[mt][:, n0:n1],
                                                    in0=ps[:, :n1 - n0],
                                                    in1=x[mt][:, n0:n1],
                                                    op=ALU.add)

            def ffn(s, l, x):
                W = WS[s]
                nch = _chunks(W, 288)
                with tc.tile_pool(name="pF", bufs=1, space="PSUM") as pF:
                    r2 = norm_r(x, W, pF, ptag="pa", pbufs=3)
                    hn = [sb.tile([128, W], BF16, name="hn", tag="xn", bufs=8)
                          for _ in range(8)]
                    for k in range(8):
                        nc.vector.tensor_tensor(out=hn[k], in0=x[k], in1=r2,
                                                op=ALU.mult)
                    for n0, n1 in nch:
                        nw = n1 - n0
                        u = sb.tile([128, 32, 288], BF16, name="u", tag="u",
                                    bufs=1)
                        w2ts = {}
                        for mt in range(2):
                            w2ts[mt] = wp.tile([128, 32, 128], BF16,
                                               name="w2t", tag="w2", bufs=2)
                            nc.sync.dma_start(w2ts[mt], D[f"w2{l}"][mt])
                        for ko in range(32):
                            w1t = wp.tile([128, 8, 128], BF16, name="w1t",
                                          tag="w1", bufs=4)
                            nc.sync.dma_start(w1t, D[f"w1{l}"][ko])
                            w3t = wp.tile([128, 8, 128], BF16, name="w3t",
                                          tag="w3", bufs=4)
                            nc.sync.dma_start(w3t, D[f"w3{l}"][ko])
                            pa = pF.tile([128, 512], F32, name="pa", tag="pa",
                                         bufs=3)[:, :288]
                            pb = pF.tile([128, 512], F32, name="pb", tag="pb",
                                         bufs=3)[:, :288]
                            for kt in range(8):
                                nc.tensor.matmul(pa[:, :nw], w1t[:, kt, :],
                                                 hn[kt][:, n0:n1],
                                                 start=(kt == 0),
                                                 stop=(kt == 7))
                            for kt in range(8):
                                nc.tensor.matmul(pb[:, :nw], w3t[:, kt, :],
                                                 hn[kt][:, n0:n1],
                                                 start=(kt == 0),
                                                 stop=(kt == 7))
                            sl = sb.tile([128, 288], BF16, name="sl", tag="sl",
                                         bufs=4)
                            nc.scalar.activation(out=sl[:, :nw],
                                                 in_=pa[:, :nw], func=AF.Silu)
                            nc.vector.tensor_tensor(out=u[:, ko, :nw],
                                                    in0=sl[:, :nw],
                                                    in1=pb[:, :nw],
                                                    op=ALU.mult)
                        for mt in range(8):
                            if mt in w2ts:
                                w2t = w2ts[mt]
                            else:
                                w2t = wp.tile([128, 32, 128], BF16, name="w2t",
                                              tag="w2", bufs=2)
                                nc.sync.dma_start(w2t, D[f"w2{l}"][mt])
                            pc = pF.tile([128, 288], F32, name="pc", tag="pc",
                                         bufs=2)
                            for ko in range(32):
                                nc.tensor.matmul(pc[:, :nw], w2t[:, ko, :],
                                                 u[:, ko, :nw],
                                                 start=(ko == 0),
                                                 stop=(ko == 31))
                            nc.vector.tensor_tensor(out=x[mt][:, n0:n1],
                                                    in0=pc[:, :nw],
                                                    in1=x[mt][:, n0:n1],
                                                    op=ALU.add)

            def upsample(s, x):
                W = WS[s]
                U = LS[s + 1] // 2
                d = DUP[s]
                xb = [sb.tile([128, W], BF16, name="xb", tag="xsq", bufs=8)
                      for _ in range(8)]
                for k in range(8):
                    nc.vector.tensor_copy(out=xb[k], in_=x[k])
                xnext = xtiles(s + 1)
                with tc.tile_pool(name="pU", bufs=1, space="PSUM") as pU:
                    for par in range(2):
                        for mt in range(8):
                            pu = pU.tile([128, 384], F32, name="pu", tag="pu",
                                         bufs=2)
                            first = True
                            for tap, shift in ((par, 0), (par + 2, -1)):
                                wt = wp.tile([128, 8, 128], BF16, name="wupt",
                                             tag="wup", bufs=3)
                                nc.sync.dma_start(wt, D[f"wup{s}"][tap, mt])
                                for kt in range(8):
                                    c0 = PADL + d + shift
                                    nc.tensor.matmul(
                                        pu[:, :U], wt[:, kt, :],
                                        xb[kt][:, c0:c0 + U],
                                        start=first,
                                        stop=(tap >= 2 and kt == 7))
                                    first = False
                            nc.vector.tensor_copy(
                                out=xnext[mt][:, PADL + par: PADL + par + 2 * U - 1: 2],
                                in_=pu[:, :U])
                return xnext

            # ---------------- stages ----------------
            layers_done = 0
            stop = False
            for s in range(4):
                if stop:
                    break
                eb0t = cst.tile([128, 8, IW], BF16, name="eb0t", tag="eb",
                                bufs=2)
                nc.sync.dma_start(eb0t, D["eb0"][s])
                eb1t = cst.tile([128, 8, IW], BF16, name="eb1t", tag="eb",
                                bufs=2)
                nc.sync.dma_start(eb1t, D["eb1"][s])
                for li in range(2):
                    if not full and layers_done >= n_layers:
                        stop = True
                        break
                    l = 2 * s + li
                    attn(s, l, x, eb0t, eb1t)
                    ffn(s, l, x)
                    layers_done += 1
                if not stop and s < 3:
                    x = upsample(s, x)

            if full:
                # out conv
                x3b = [sb.tile([128, WS[3]], BF16, name="x3b", tag="xsq",
                               bufs=8) for _ in range(8)]
                for k in range(8):
                    nc.vector.tensor_copy(out=x3b[k], in_=x[k])
                out_sb = sb.tile([128, 2, 512], F32, name="out_sb",
                                 tag="out_sb", bufs=1)
                with tc.tile_pool(name="pO", bufs=1, space="PSUM") as pO:
                    for mt in range(2):
                        ps = pO.tile([128, 512], F32, name="pso", tag="pout",
                                     bufs=2)
                        first = True
                        for r in range(7):
                            wt = wp.tile([128, 8, 128], BF16, name="woutt",
                                         tag="wup", bufs=3)
                            nc.sync.dma_start(wt, D["wout"][r, mt])
                            for kt in range(8):
                                c0 = OUTC0 - 6 + r
                                nc.tensor.matmul(ps, wt[:, kt, :],
                                                 x3b[kt][:, c0:c0 + 512],
                                                 start=first,
                                                 stop=(r == 6 and kt == 7))
                                first = False
                        nc.vector.tensor_copy(out=out_sb[:, mt, :], in_=ps)
                nc.sync.dma_start(out_d[0], out_sb[:, 0, :])
                nc.sync.dma_start(out_d[1], out_sb[:, 1, :])
            else:
                dbg_sb = sb.tile([128, 8, WS[dbg_stage]], F32, name="dbg",
                                 tag="dbg", bufs=1)
                for k in range(8):
                    nc.vector.tensor_copy(out=dbg_sb[:, k, :], in_=x[k])
                for k in range(8):
                    nc.sync.dma_start(out_d[k], dbg_sb[:, k, :])

    _split_multiwait(nc)
    return nc


# ---------------------------------------------------------------------------
# public entry point
# ---------------------------------------------------------------------------

_NC_CACHE = {}


def _get_nc(n_layers=8, dbg_stage=None):
    key = (n_layers, dbg_stage)
    if key not in _NC_CACHE:
        _NC_CACHE[key] = _build(n_layers, dbg_stage)
    return _NC_CACHE[key]


def run(codes, params, n_layers=8, dbg_stage=None):
    """Run the kernel; returns list of per-core 'out' arrays (raw)."""
    nc = _get_nc(n_layers, dbg_stage)
    shared = _prep_shared(params)
    per_core = _prep_percore(codes, params)
    in_maps = [{**shared, **pc} for pc in per_core]
    res = run_bass_kernel_spmd(nc, in_maps, core_ids=list(range(NCORE)))
    return [r["out"] for r in res.results]


def kernel(codes, params):
    outs = run(codes, params)
    y = np.zeros((B, 2048, PATCH), np.float32)
    for core in range(NCORE):
        b, c = divmod(core, 4)
        o = np.asarray(outs[core], np.float32).reshape(256, 512)[:PATCH]
        y[b, 512 * c: 512 * (c + 1), :] = o.T
    return y.reshape(B, -1)
